# revision 36
# baseline (speedup 1.0000x reference)
"""Trainium2 Bass kernel for CosineSimilarityWeightedAverage.

reference:
  input [B=4, C=4096, D=64] f32
  in_n = input / ||input||_row
  cos  = in_n @ in_n.T per batch            [B, C, C]
  attn = softmax(cos / 0.1, axis=-1)
  out  = (attn @ weight) * weight_global * input + (attn @ bias) * bias_global

Sharding: 8 cores = (batch b = core//2) x (query half h = core%2, 2048 rows).
Each core gets all 4096 keys of its batch (key halves swapped for h=1 so the
core's queries are always key rows 0..2048) and computes 2048 output rows.

Per-core design (fp8 DoubleRow matmuls + 3-engine exp split):
  - keys normalized on DVE (Quake rsqrt + 2 Newton steps), kn8 = fp8e4m3 of
    10*k/||k||; knT8/qnT8 built by PE transposes with the PSUM->SBUF copies
    done by HWDGE DMA (compute engines untouched).
  - stage 1: DR matmul with a 65th contraction row carrying a +52 bias so the
    exp bit-trick needs no separate add: st = kn.kn + 52 (= 100*cos + 52).
    The DoubleRow second subtile is zeroed on the rhs (qnT8), so it doubles
    matmul throughput (0.5 cyc/row) without a real second k-subtile.
  - exp split across three engines per (qc, j) chunk [128, 2, 512]:
      ACT : e = exp(0.1*st - 10.4466) in fp8e4m3 (table exp)
      DVE : int8 bits = max(st * (0.4/ln2), 0), bitcast fp8e5m2
      Pool: same bit-trick on GPSIMD
    The bit-trick is Schraudolph's exponent-bit approximation evaluated
    directly in e5m2 bit space; the ACT bias is tuned so both chunk kinds
    carry the same scale (uniform per-column factors cancel in softmax).
  - stage 2: DR matmuls, lhsT = wcat8 fp8e4m3 [128, 2, 128] per kt pair
    accumulating (attn @ [W|b])^T, plus a DR ones matmul for the denominator.
  - finalize per 512-query chunk: recip(den), partition-broadcast, normalize,
    PE transpose back to [q, d], out = avgW*(wg*x) + avgB*bg, chunked DMA out.
"""

import numpy as np

B = 4
C = 4096
D = 64
NCORES = 8
CQ = C // 2          # queries per core
KT = C // 128        # 32 k-tiles
QT = CQ // 128       # 16 query "tt" tiles in the m-order layout
NJ = KT // 2         # 16 k-tile pairs

# exp constants (see module docstring)
BIAS_V = 7.0                       # bias contraction-row value; B = v^2 = 49
A_BITS = 0.4 / float(np.log(2.0))  # int8-bit-trick multiplier
ACT_BIAS = -10.4468                # -5.2 - 0.1*49 + ln(geom-mean match)

# exp engine split across the 64 (qc, j) chunks: ACT/DVE (Pool cannot
# read PSUM, so it takes the SBUF-side work instead).  DVE positions are
# spread evenly; qc0 is lighter (DVE also does the knT copies there) and
# qc3 ends on ACT so DVE is free for the tail finalize.
DVE_EXP_JS = {
    0: (0, 2, 4, 6, 9, 11, 13),
    1: (1, 3, 6, 8, 11, 13),
    2: (1, 6, 8, 11, 13),
    3: (1, 3, 6, 8, 10, 12, 14),
}

_CACHE = {}

# m-order permutation: sbuf row (mm, tt) = partition mm, free index tt holds
# query j = (mm%64)*32 + 2*tt + (mm//64).  PERM[i] = original row of
# permuted row i (i = mm*16 + tt).
_I = np.arange(CQ)
_PERM = ((_I // QT) % 64) * 32 + 2 * (_I % QT) + (_I // QT) // 64
# accs/wbT free-index order: i = qc*512 + tk_rel*64 + c -> query c*32 + 8*qc
# + tk_rel (the stage-2 accumulator's moving-dim layout)
_QPERM = ((_I % 64) * 32 + 8 * (_I // 512) + (_I // 64) % 8)


def _exp_schedule():
    """64-long engine assignment, 'A' or 'D'."""
    sched = []
    for qc in range(4):
        js = DVE_EXP_JS[qc]
        sched += ["D" if j in js else "A" for j in range(16)]
    return sched


def _build():
    import concourse.bass as bass
    import concourse.bacc as bacc
    import concourse.mybir as mybir
    import concourse.tile as tile
    from concourse.masks import make_identity

    f32 = mybir.dt.float32
    f16 = mybir.dt.float16
    f8 = mybir.dt.float8e4
    f8e5 = mybir.dt.float8e5
    i8 = mybir.dt.int8
    i32 = mybir.dt.int32
    AF = mybir.ActivationFunctionType
    DR = mybir.MatmulPerfMode.DoubleRow
    Alu = mybir.AluOpType

    nc = bacc.Bacc(None, target_bir_lowering=False)
    xk = nc.dram_tensor("xk", [C, D], f32, kind="ExternalInput")
    zrok = nc.dram_tensor("zrok", [65, 2, KT, 128], f8, kind="ExternalInput")
    wcat8 = nc.dram_tensor("wcat8", [C, 2 * D], f8, kind="ExternalInput")
    xqp = nc.dram_tensor("xqp", [CQ, D], f32, kind="ExternalInput")
    wgp = nc.dram_tensor("wgp", [CQ, D], f32, kind="ExternalInput")
    bgp = nc.dram_tensor("bgp", [CQ, D], f16, kind="ExternalInput")
    out = nc.dram_tensor("out", [CQ, D], f32, kind="ExternalOutput")

    sched = _exp_schedule()

    with tile.TileContext(nc) as tc:
        with (
            tc.tile_pool(name="singles", bufs=1) as singles,
            tc.tile_pool(name="sb", bufs=2) as sb,
            tc.tile_pool(name="e8p", bufs=8) as e8p,
            tc.tile_pool(name="fin", bufs=4) as fin,
            tc.tile_pool(name="stage", bufs=3, space="PSUM") as stage,
            tc.tile_pool(name="acc", bufs=1, space="PSUM") as accp,
            tc.tile_pool(name="den", bufs=1, space="PSUM") as denp,
        ):
            # ---------------- transposed-operand homes + constant regions ---
            # knT8: [65, 2(sub), KT, 128] — contraction rows 0..63 = d,
            # row 64 = bias B (subtile 0).  Subtile 1 is never read as nonzero
            # because qnT8's subtile 1 is zeroed.
            knT8 = singles.tile([65, 2, KT, 128], f8)

            identity16 = singles.tile([128, 128], f16)
            make_identity(nc, identity16)
            ones8 = singles.tile([128, 2, 128], f8)
            nc.gpsimd.memset(ones8, 1.0)
            ebias = singles.tile([128, 1], f32)
            nc.gpsimd.memset(ebias, ACT_BIAS)
            # dummy activation: forces the Exp table load onto ACT now,
            # during the input DMAs, instead of before the first real exp
            warm = singles.tile([128, 1], f32)
            nc.scalar.activation(out=warm, in_=ebias, func=AF.Exp)

            # ---------------- loads ----------------
            xk_r = xk.rearrange("(p t) d -> p t d", p=128)
            kbig = singles.tile([128, KT, D], f32)
            nc.sync.dma_start(out=kbig[:, 0:4, :], in_=xk_r[:, 0:4, :])
            nc.sync.dma_start(out=kbig[:, 4:8, :], in_=xk_r[:, 4:8, :])
            for c in range(1, 4):
                cs = slice(8 * c, 8 * (c + 1))
                nc.sync.dma_start(out=kbig[:, cs, :], in_=xk_r[:, cs, :])
            wsb8 = singles.tile([128, KT, 2 * D], f8)
            nc.sync.dma_start(
                out=wsb8, in_=wcat8.rearrange("(p t) m -> p t m", p=128)
            )
            # constant region comes in by DMA (host-prepared) so no engine
            # pays for the fill: whole-tile zero init + bias row (= v, so a
            # score picks up v*v = 49 from row 64 of both operands).  Emitted
            # after the kbig chunks so the norm chain starts immediately.
            nc.sync.dma_start(out=knT8, in_=zrok[:, :, :, :])
            xqs = singles.tile([128, QT, D], f32)
            nc.sync.dma_start(out=xqs, in_=xqp.rearrange("(p t) d -> p t d", p=128))
            wgs = singles.tile([128, QT, D], f32)
            nc.sync.dma_start(out=wgs, in_=wgp.rearrange("(p t) d -> p t d", p=128))
            bgs = singles.tile([128, QT, D], f16)
            nc.sync.dma_start(out=bgs, in_=bgp.rearrange("(p t) d -> p t d", p=128))

            # ---------------- k norms (DVE only), 4-chunk pipeline ----------
            # Per 8-tile chunk: square+reduce, Quake rsqrt + 2 Newton steps
            # (DVE-only so ACT stays exp-exclusive), normalize straight to
            # fp8e4m3, and transpose immediately so the first stage-1 matmul
            # isn't gated on the full norm chain.
            ksq = singles.tile([128, KT], f32)
            iv = singles.tile([128, KT], i32)
            iv2 = singles.tile([128, KT], i32)
            ya = singles.tile([128, KT], f32)
            yb = singles.tile([128, KT], f32)
            kscale = singles.tile([128, KT], f32)
            kn16 = singles.tile([128, KT, D], f16)

            def emit_rsqrt_chain(cs):
                nc.vector.tensor_scalar(
                    out=iv[:, cs], in0=ksq[:, cs].bitcast(i32), scalar1=1,
                    scalar2=None, op0=Alu.logical_shift_right,
                )
                nc.vector.tensor_scalar(
                    out=iv2[:, cs], in0=iv[:, cs], scalar1=-1,
                    scalar2=0x5F3759DF, op0=Alu.mult, op1=Alu.add,
                )
                y0 = iv2[:, cs].bitcast(f32)
                # single Newton step with the temperature x10 folded in:
                # kscale = 10*y0*(1.5 - 0.5*s*y0^2) = y0*(15 - 5*s*y0^2).
                # (~0.2% max error - far below the fp8e4m3 quantization)
                nc.vector.tensor_mul(ya[:, cs], y0, y0)
                nc.vector.tensor_mul(yb[:, cs], ya[:, cs], ksq[:, cs])
                nc.vector.tensor_scalar(
                    out=yb[:, cs], in0=yb[:, cs], scalar1=-5.0, scalar2=15.0,
                    op0=Alu.mult, op1=Alu.add,
                )
                nc.vector.tensor_mul(kscale[:, cs], y0, yb[:, cs])

            def emit_norm_half(h):
                # chunk 0 in two 4-tile halves, all on DVE, lowest latency
                hs = slice(4 * h, 4 * h + 4)
                ktmp = sb.tile([128, 4, D], f32, tag="ktmp", name=f"ktmp0{h}")
                nc.vector.tensor_mul(ktmp, kbig[:, hs, :], kbig[:, hs, :])
                nc.vector.reduce_sum(
                    out=ksq[:, hs], in_=ktmp, axis=mybir.AxisListType.X
                )
                emit_rsqrt_chain(hs)
                for t in range(4 * h, 4 * h + 4):
                    nc.vector.tensor_scalar_mul(
                        out=kn16[:, t, :], in0=kbig[:, t, :],
                        scalar1=kscale[:, t : t + 1],
                    )
                emit_knT_group(h)

            def emit_norm_chunk(c):
                cs = slice(8 * c, 8 * (c + 1))
                ktmp = sb.tile([128, 8, D], f32, tag="ktmp", name=f"ktmp{c}")
                nc.gpsimd.tensor_mul(ktmp, kbig[:, cs, :], kbig[:, cs, :])
                nc.vector.reduce_sum(
                    out=ksq[:, cs], in_=ktmp, axis=mybir.AxisListType.X
                )
                if c == 3:
                    # one batched chain for chunks 1..3 (fewer, wider DVE ops)
                    emit_rsqrt_chain(slice(8, 32))
                    for t in range(8, 32):
                        nc.gpsimd.tensor_scalar_mul(
                            out=kn16[:, t, :], in0=kbig[:, t, :],
                            scalar1=kscale[:, t : t + 1],
                        )

            # ---------------- transposed layouts ----------------
            # knT8[0:64, 0, t, :] = kn8[:, t, :].T   [64, 128] per k-tile
            # qnT8[0:64, 0, tk, :] = kn8[0:64, tk, :].T  [64, 64] per q tile
            # (queries are key rows 0..2048 = partitions 0..64, all tk)
            # PE transposes into PSUM, then HWDGE DMA to SBUF in 4-tile
            # batches so no compute engine pays for the copy.
            def emit_knT_group(g):  # g in 0..7, tiles 4g..4g+3
                pt = stage.tile([64, 4, 128], f16, tag="stage", name=f"ptk{g}")
                for i in range(4):
                    nc.tensor.transpose(
                        pt[:, i, :], kn16[:, 4 * g + i, :], identity16
                    )
                nc.scalar.activation(
                    out=knT8[0:64, 0, 4 * g : 4 * g + 4, :], in_=pt,
                    func=AF.Copy,
                )

            emit_norm_half(0)
            emit_norm_half(1)
            for c in range(1, 4):
                emit_norm_chunk(c)

            winp = singles.tile([128, QT, D], f16)

            # ---------------- main loop ----------------
            out_nat = singles.tile([128, QT, D], f32)
            out_r = out.rearrange("(p t) d -> p t d", p=128)
            acc_ps = {}
            den_ps = {}
            pending = []

            def flush_one():
                qc, j, e8 = pending.pop(0)
                nc.tensor.matmul(
                    acc_ps[qc], lhsT=wsb8[:, 2 * j : 2 * j + 2, :], rhs=e8,
                    start=(j == 0), stop=(j == NJ - 1),
                    perf_mode=DR, skip_group_check=True,
                )
                nc.tensor.matmul(
                    den_ps[qc], lhsT=ones8, rhs=e8,
                    start=(j == 0), stop=(j == NJ - 1),
                    perf_mode=DR, skip_group_check=True,
                )

            fin_state = {}

            def finalize_a(qc):
                # den matmul wrote the denominator to all 128 partitions, so
                # no partition broadcast is needed.
                rinv = fin.tile([128, 512], f32, tag="rinv", name=f"rinv{qc}")
                nc.vector.reciprocal(out=rinv, in_=den_ps[qc])
                accs = fin.tile([128, 512], f16, tag="accs", name=f"accs{qc}")
                nc.vector.tensor_mul(accs, acc_ps[qc], rinv)
                fin_state[qc] = accs

            def finalize_b(qc):
                accs = fin_state[qc]
                ot4 = denp.tile([128, 4, 128], f16, tag="den",
                                name=f"ot{qc}")
                for ms in range(4):
                    nc.tensor.transpose(
                        ot4[:, ms, :], accs[:, ms * 128 : (ms + 1) * 128],
                        identity16,
                    )
                fin_state[qc] = ot4

            def finalize_c(qc):
                ot4 = fin_state.pop(qc)
                tt = slice(4 * qc, 4 * qc + 4)
                t1 = fin.tile([128, 4, D], f16, tag="t1", name=f"t1_{qc}")
                nc.vector.tensor_mul(t1, ot4[:, :, 0:64], winp[:, tt, :])
                t2 = fin.tile([128, 4, D], f16, tag="t2", name=f"t2_{qc}")
                nc.vector.tensor_mul(t2, ot4[:, :, 64:128], bgs[:, tt, :])
                nc.vector.tensor_add(out_nat[:, tt, :], t1, t2)
                nc.sync.dma_start(out=out_r[:, tt, :], in_=out_nat[:, tt, :])

            def finalize(qc):
                finalize_a(qc)
                finalize_b(qc)
                finalize_c(qc)

            for qc in range(4):
                acc_ps[qc] = accp.tile([128, 512], f32, tag="acc", name=f"acc{qc}")
                den_ps[qc] = denp.tile([128, 512], f32, tag="den", name=f"den{qc}")
                # queries are key rows 0..2047 = the first 64 columns of
                # each knT tile, so the rhs is just a strided view of knT8
                rhs = knT8[:, :, 8 * qc : 8 * qc + 8, 0:64]
                for j in range(NJ):
                    if qc == 0 and j <= 5:
                        # prefetch remaining transposes ahead of use
                        emit_knT_group(j + 2)
                    if qc == 0 and j == 7:
                        # winp = wg * x per query row; needed from finalize(0)
                        nc.gpsimd.tensor_mul(winp, wgs, xqs)
                    st = stage.tile([128, 2, 512], f32, tag="stage",
                                    name=f"st{qc}_{j}")
                    for par in range(2):
                        nc.tensor.matmul(
                            st[:, par, :], lhsT=knT8[:, :, 2 * j + par, :],
                            rhs=rhs, start=True, stop=True, perf_mode=DR,
                        )
                    eng = sched[16 * qc + j]
                    if eng == "A":
                        e8 = e8p.tile([128, 2, 512], f8, tag="e8",
                                      name=f"e{qc}_{j}")
                        nc.scalar.activation(out=e8, in_=st, func=AF.Exp,
                                             scale=0.1, bias=ebias[:, 0:1])
                    else:
                        e8 = e8p.tile([128, 2, 512], f8e5, tag="e8",
                                      name=f"e{qc}_{j}")
                        veng = nc.vector if eng == "D" else nc.gpsimd
                        veng.tensor_scalar(
                            out=e8.bitcast(i8), in0=st, scalar1=A_BITS,
                            scalar2=0.0, op0=Alu.mult, op1=Alu.max,
                        )
                    pending.append((qc, j, e8))
                    if qc > 0 and j == 3:
                        finalize(qc - 1)
                    lag = 1 if (qc == 3 and j >= 13) else 3
                    while len(pending) > lag:
                        flush_one()
            while pending:
                flush_one()
            finalize(3)

    nc.compile()
    return nc


def _get_nc():
    if "nc" not in _CACHE:
        _CACHE["nc"] = _build()
    return _CACHE["nc"]


def _make_in_maps(input, weight, bias, weight_global, bias_global):
    import ml_dtypes

    input = np.ascontiguousarray(np.asarray(input, dtype=np.float32))
    ones = lambda: np.ones((C, D), np.float32)
    weight = ones() if weight is None else np.asarray(weight, np.float32)
    bias = np.zeros((C, D), np.float32) if bias is None else np.asarray(bias, np.float32)
    weight_global = ones() if weight_global is None else np.asarray(weight_global, np.float32)
    bias_global = ones() if bias_global is None else np.asarray(bias_global, np.float32)
    wcat8 = np.concatenate([weight, bias], axis=1).astype(ml_dtypes.float8_e4m3)
    zrok = np.zeros((65, 2, KT, 128), ml_dtypes.float8_e4m3)
    zrok[64, 0] = ml_dtypes.float8_e4m3(BIAS_V)
    wcat8_sw = np.ascontiguousarray(np.concatenate([wcat8[CQ:], wcat8[:CQ]]))
    wcat8 = np.ascontiguousarray(wcat8)
    in_maps = []
    for core in range(NCORES):
        b, h = divmod(core, 2)
        sl = slice(h * CQ, (h + 1) * CQ)
        if h == 0:
            xk_c, wc_c = input[b], wcat8
        else:
            xk_c = np.concatenate([input[b, CQ:], input[b, :CQ]])
            wc_c = wcat8_sw
        in_maps.append({
            "xk": np.ascontiguousarray(xk_c),
            "wcat8": wc_c,
            "zrok": zrok,
            "xqp": np.ascontiguousarray(input[b, sl][_PERM]),
            "wgp": np.ascontiguousarray(weight_global[sl][_PERM]),
            "bgp": np.ascontiguousarray(
                bias_global[sl][_PERM].astype(np.float16)),
        })
    return in_maps


def _run(in_maps, **kw):
    from concourse.bass_utils import run_bass_kernel_spmd
    nc = _get_nc()
    return run_bass_kernel_spmd(nc, in_maps, core_ids=list(range(NCORES)), **kw)


def kernel(input, weight=None, bias=None, weight_global=None, bias_global=None,
           **_ignored):
    in_maps = _make_in_maps(input, weight, bias, weight_global, bias_global)
    res = _run(in_maps)
    out = np.empty((B, C, D), np.float32)
    for core in range(NCORES):
        b, h = divmod(core, 2)
        out[b, h * CQ + _PERM] = res.results[core]["out"]
    return out
